# revision 1
# baseline (speedup 1.0000x reference)
"""Trainium2 Bass kernel for a char-GRU:
  y = FC(GRU_last_hidden(Embed(x)))   with V=128, E=H=OUT=768, B=128, T=512.

Strategy (per core, data-parallel over batch, 8 cores x 16 rows):
  - table[v, :] = emb[v] @ W_ih.T + b_ih (+ b_hh for the r/z gate columns),
    computed on-device once.  Since V=128, the big input-side GEMM
    xe @ W_ih.T collapses into a row-gather from this [128, 2304] table.
  - The gather is done on the tensor engine: a one-hot [128v, 16b] stationary
    tile accumulates table rows directly into the gate PSUM banks.
  - 512 sequential GRU steps; per step the moving operand is W_hh^T
    (fp32r, 1 col/cycle), stationary is h^T (16 cols, cheap reload).
  - h_new = h + (1-z)*(n-h); (1-z) computed directly as sigmoid(-pre_z).
  - h_new [16, 768] is transposed back to h^T via 6 PE transposes.
"""

import os
import numpy as np
from contextlib import ExitStack

import concourse.bass as bass
import concourse.bacc as bacc
import concourse.tile as tile
from concourse import mybir
from concourse.bass_utils import run_bass_kernel_spmd

F32 = mybir.dt.float32
F32R = mybir.dt.float32r
I32 = mybir.dt.int32

V, E, H, OUT = 128, 768, 768, 768
G3 = 3 * H           # 2304
B_FULL, T_FULL = 128, 512
NCORES = 8
BS = B_FULL // NCORES  # 16
KT = H // 128          # 6 hidden k-tiles


def _bank_chunks(start, length):
    """Split [start, start+length) into pieces not crossing 512-elem banks."""
    cur, end = start, start + length
    while cur < end:
        w = min(512 - (cur % 512), end - cur)
        yield cur, w
        cur += w


def emit_kernel(ctx: ExitStack, tc: tile.TileContext, io: dict, T: int,
                REPS: int = 1):
    nc = tc.nc
    add = mybir.AluOpType.add
    sub = mybir.AluOpType.subtract
    mult = mybir.AluOpType.mult
    iseq = mybir.AluOpType.is_equal
    Sig = mybir.ActivationFunctionType.Sigmoid
    Tanh = mybir.ActivationFunctionType.Tanh

    oh_d, whhT_d, bhh_d, table_d, fcT_d, fcb_d, y_d = (
        io["oh"], io["whhT"], io["bhh"], io["table"], io["fcT"],
        io["fcb"], io["y"],
    )

    consts = ctx.enter_context(tc.tile_pool(name="consts", bufs=1))

    # ---- persistent SBUF ----
    whhT_sb = consts.tile([128, KT, G3], F32R, name="whhT_sb")
    table_sb = consts.tile([128, G3], F32R, name="table_sb")
    onehot_sb = consts.tile([128, T * BS], F32R, name="onehot_sb")
    fcT_sb = consts.tile([128, KT, OUT], F32R, name="fcT_sb")
    fcb_sb = consts.tile([1, OUT], F32R, name="fcb_sb")
    bhh_sb = consts.tile([1, G3], F32R, name="bhh_sb")
    ones1b = consts.tile([1, BS], F32R, name="ones1b")
    ident16 = consts.tile([BS, BS], F32, name="ident16")
    ones16 = consts.tile([BS, BS], F32, name="ones16")

    # ---- step state ----
    state = ctx.enter_context(tc.tile_pool(name="state", bufs=1))
    h_pp = [state.tile([BS, H], F32, name=f"h_{i}") for i in range(2)]
    hT_pp = [state.tile([128, KT * BS], F32R, name=f"hT_{i}") for i in range(2)]

    tmp = ctx.enter_context(tc.tile_pool(name="tmp", bufs=2))
    ps = ctx.enter_context(tc.tile_pool(name="ps", bufs=1, space="PSUM"))

    def emit_init():
        """Per-run init: pure DMAs (table + one-hots precomputed on host)
        plus two tiny const builds."""
        for k in range(KT):
            nc.sync.dma_start(whhT_sb[:, k, :], whhT_d[k])
            nc.sync.dma_start(fcT_sb[:, k, :], fcT_d[k])
        nc.sync.dma_start(bhh_sb[:], bhh_d[:])
        nc.sync.dma_start(fcb_sb[:], fcb_d[:])
        nc.sync.dma_start(table_sb[:], table_d[:])
        # one-hot matrix, t-major: [V, T*BS]; split DMA for queue overlap
        nq = 4
        step = (T * BS) // nq
        for q in range(nq):
            nc.sync.dma_start(onehot_sb[:, q * step:(q + 1) * step],
                              oh_d[:, q * step:(q + 1) * step])
        nc.vector.memset(ones16[:], 1.0)
        nc.scalar.copy(ones1b[:], ones16[0:1, :].bitcast(F32R))
        # identity[p, f] = 1.0 where f == p
        nc.gpsimd.affine_select(ident16[:], ones16[:], pattern=[[1, BS]],
                                compare_op=iseq, fill=0.0, base=0,
                                channel_multiplier=-1)

    def emit_body():
      emit_init()

      def g_xn0(t, g_n, gfirst):
        oh = onehot_sb[:, t * BS:(t + 1) * BS]
        nc.tensor.matmul(g_n[:, 512:1024], oh, table_sb[:, 2 * H:2 * H + 512],
                         start=True, stop=True)

      def g_xn1(t, g_n, gfirst):
        oh = onehot_sb[:, t * BS:(t + 1) * BS]
        nc.tensor.matmul(g_n[:, 1024:1280], oh,
                         table_sb[:, 2 * H + 512:3 * H],
                         start=True, stop=True)

      def g_rzg(t, g_rz, gfirst):
        oh = onehot_sb[:, t * BS:(t + 1) * BS]
        for c0, w in _bank_chunks(0, 2 * H):
            nc.tensor.matmul(g_rz[:, c0:c0 + w], oh,
                             table_sb[:, c0:c0 + w], start=True, stop=gfirst)

      def g_bias0(g_n, gfirst):
        nc.tensor.matmul(g_n[:, 0:512], ones1b[:],
                         bhh_sb[:, 2 * H:2 * H + 512], start=True, stop=gfirst)

      def g_bias1(g_n, gfirst):
        nc.tensor.matmul(g_n[:, 1280:1536], ones1b[:],
                         bhh_sb[:, 2 * H + 512:3 * H], start=True, stop=gfirst)

      def emit_gather(t, g_rz, g_n, gfirst):
        """x-side gate gathers + b_hh(n) bias rows for step t -- independent
        of h^T(t-1), so these can stream while step t-1's gate tail runs."""
        g_xn0(t, g_n, gfirst)
        g_xn1(t, g_n, gfirst)
        g_rzg(t, g_rz, gfirst)
        g_bias0(g_n, gfirst)
        g_bias1(g_n, gfirst)

      def alloc_ps():
        g_rz = ps.tile([BS, 2 * H], F32, name="ps_rz", tag="rz")
        # [hn0 (512) | xn (768) | hn1 (256)] -- each matmul group in-bank
        g_n = ps.tile([BS, 1536], F32, name="ps_n", tag="nx")
        return g_rz, g_n

      for t in range(T):
        first = (t == 0)
        h_prev, h_new = h_pp[(t + 1) % 2], h_pp[t % 2]
        hT_prev, hT_new = hT_pp[(t + 1) % 2], hT_pp[t % 2]

        ps_rz, ps_n = alloc_ps()
        ps_hT = ps.tile([128, KT * BS], F32, name="ps_hT", tag="ht")
        emit_gather(t, ps_rz, ps_n, first)

        r_t = tmp.tile([BS, H], F32, name="r_t", tag="r")
        z_t = tmp.tile([BS, H], F32, name="z_t", tag="z")
        u_t = tmp.tile([BS, H], F32, name="u_t", tag="u")
        a_t = tmp.tile([BS, H], F32, name="a_t", tag="a")
        b_t = tmp.tile([BS, H], F32, name="b_t", tag="b")
        n_t = tmp.tile([BS, H], F32, name="n_t", tag="n")
        d1_t = tmp.tile([BS, H], F32, name="d1_t", tag="d1")
        d2_t = tmp.tile([BS, H], F32, name="d2_t", tag="d2")

        def kloop(ps_t, c0, w, g0):
            if first:
                return
            for k in range(KT):
                nc.tensor.matmul(ps_t[:, c0:c0 + w],
                                 hT_prev[:, k * BS:(k + 1) * BS],
                                 whhT_sb[:, k, g0:g0 + w],
                                 start=False, stop=(k == KT - 1))

        # recurrent matmuls chunk-by-chunk, with the gate chain
        # interleaved so ACT/DVE work overlaps the MM stream
        kloop(ps_rz, 0, 512, 0)
        kloop(ps_rz, 512, 512, 512)
        nc.scalar.activation(r_t[:], ps_rz[:, 0:H], Sig)
        kloop(ps_n, 0, 512, 2 * H)              # hn[0:512]
        kloop(ps_rz, 1024, 512, 1024)
        nc.scalar.activation(z_t[:], ps_rz[:, H:2 * H], Sig)
        if not first:
            # d2 = z*h on the otherwise idle gpsimd engine, in halves
            nc.gpsimd.tensor_tensor(d2_t[:, 0:512], z_t[:, 0:512],
                                    h_prev[:, 0:512], mult)
            nc.gpsimd.tensor_tensor(d2_t[:, 512:768], z_t[:, 512:768],
                                    h_prev[:, 512:768], mult)
        kloop(ps_n, 1280, 256, 2 * H + 512)     # hn[512:768]
        nc.vector.tensor_tensor(a_t[:, 0:512], r_t[:, 0:512],
                                ps_n[:, 0:512], mult)
        nc.vector.tensor_tensor(b_t[:, 0:512], a_t[:, 0:512],
                                ps_n[:, 512:1024], add)
        nc.scalar.activation(n_t[:, 0:512], b_t[:, 0:512], Tanh)
        nc.vector.tensor_tensor(a_t[:, 512:768], r_t[:, 512:768],
                                ps_n[:, 1280:1536], mult)
        nc.vector.tensor_tensor(b_t[:, 512:768], a_t[:, 512:768],
                                ps_n[:, 1024:1280], add)
        nc.vector.tensor_scalar(u_t[:], z_t[:], -1.0, 1.0, mult, add)
        nc.scalar.activation(n_t[:, 512:768], b_t[:, 512:768], Tanh)
        # update halves; h^T transposes batched into one psum tile
        for h0, hw_ in ((0, 512), (512, 256)):
            sl = slice(h0, h0 + hw_)
            if first:
                nc.vector.tensor_tensor(h_new[:, sl], u_t[:, sl],
                                        n_t[:, sl], mult)
            else:
                nc.vector.tensor_tensor(d1_t[:, sl], u_t[:, sl],
                                        n_t[:, sl], mult)
                nc.vector.tensor_tensor(h_new[:, sl], d1_t[:, sl],
                                        d2_t[:, sl], add)
            for k in range(h0 // 128, (h0 + hw_) // 128):
                nc.tensor.transpose(ps_hT[:, k * BS:(k + 1) * BS],
                                    h_new[:, k * 128:(k + 1) * 128],
                                    ident16[:])
            nc.scalar.copy(hT_new[:, h0 // 128 * BS:(h0 + hw_) // 128 * BS],
                           ps_hT[:, h0 // 128 * BS:(h0 + hw_) // 128 * BS])

      # ---- FC head: y = h_T @ fc_W^T + fc_b ----
      hT_last = hT_pp[(T - 1) % 2]
      y_sb = consts.tile([BS, OUT], F32, name="y_sb")
      for c0 in range(0, OUT, 512):
        w = min(512, OUT - c0)
        ps_fc = ps.tile([BS, 512], F32, name="ps_fc", tag="rz")
        nc.tensor.matmul(ps_fc[:, 0:w], ones1b[:], fcb_sb[:, c0:c0 + w],
                         start=True, stop=False)
        for k in range(KT):
            nc.tensor.matmul(ps_fc[:, 0:w], hT_last[:, k * BS:(k + 1) * BS],
                             fcT_sb[:, k, c0:c0 + w],
                             start=False, stop=(k == KT - 1))
        nc.scalar.copy(y_sb[:, c0:c0 + w], ps_fc[:, 0:w])
      nc.sync.dma_start(y_d[:], y_sb[:])

    # Each rep is a complete, independent run (h reset at t=0), so the
    # marginal time per iteration of this loop is the per-run exec time.
    if REPS == 1:
        emit_body()
    else:
        with tc.For_i(0, REPS, 1):
            emit_body()


def build(T: int = T_FULL, num_devices: int = NCORES, reps: int = 1):
    nc = bacc.Bacc("TRN2", target_bir_lowering=False, debug=False,
                   enable_asserts=False, num_devices=num_devices)
    io = {
        "oh": nc.dram_tensor("oh", [128, T * BS], F32R,
                             kind="ExternalInput").ap(),
        "whhT": nc.dram_tensor("whhT", [KT, 128, G3], F32R,
                               kind="ExternalInput").ap(),
        "bhh": nc.dram_tensor("bhh", [1, G3], F32R, kind="ExternalInput").ap(),
        "table": nc.dram_tensor("table", [128, G3], F32R,
                                kind="ExternalInput").ap(),
        "fcT": nc.dram_tensor("fcT", [KT, 128, OUT], F32R,
                              kind="ExternalInput").ap(),
        "fcb": nc.dram_tensor("fcb", [1, OUT], F32R, kind="ExternalInput").ap(),
        "y": nc.dram_tensor("y", [BS, OUT], F32, kind="ExternalOutput").ap(),
    }
    with tile.TileContext(nc) as tc, ExitStack() as ctx:
        emit_kernel(ctx, tc, io, T, REPS=reps)
    nc.compile()
    return nc


def make_in_maps(x, emb, W_ih, W_hh, b_ih, b_hh, fc_W, fc_b,
                 T: int = T_FULL, ncores: int = NCORES):
    x = np.asarray(x).astype(np.int32)[:, :T]
    emb = np.asarray(emb, np.float32)
    W_ih = np.asarray(W_ih, np.float32)
    b_ih = np.asarray(b_ih, np.float32)
    b_hh = np.asarray(b_hh, np.float32)
    whhT = np.ascontiguousarray(np.asarray(W_hh, np.float32).T).reshape(
        KT, 128, G3)
    fcT = np.ascontiguousarray(np.asarray(fc_W, np.float32).T).reshape(
        KT, 128, OUT)
    bhh = b_hh.reshape(1, G3)
    fcb = np.asarray(fc_b, np.float32).reshape(1, OUT)
    # gate table: row v = emb[v] @ W_ih.T + b_ih (+ b_hh on r/z columns)
    table = emb @ W_ih.T + b_ih
    table[:, :2 * H] += b_hh[:2 * H]
    table = np.ascontiguousarray(table, np.float32)      # [V, 3H]
    shared = {"whhT": whhT, "bhh": bhh, "table": table, "fcT": fcT,
              "fcb": fcb}
    cols = np.arange(T * BS)
    in_maps = []
    for c in range(ncores):
        xs = x[c * BS:(c + 1) * BS]                       # [BS, T]
        x_tmaj = np.ascontiguousarray(xs.T).reshape(T * BS)  # t-major
        oh = np.zeros((V, T * BS), np.float32)
        oh[x_tmaj, cols] = 1.0
        in_maps.append({"oh": oh, **shared})
    return in_maps


_CACHE = {}


def kernel(x, emb, W_ih, W_hh, b_ih, b_hh, fc_W, fc_b):
    if "nc" not in _CACHE:
        _CACHE["nc"] = build()
    nc = _CACHE["nc"]
    in_maps = make_in_maps(x, emb, W_ih, W_hh, b_ih, b_hh, fc_W, fc_b)
    res = run_bass_kernel_spmd(nc, in_maps, core_ids=list(range(NCORES)))
    y = np.concatenate([res.results[c]["y"] for c in range(NCORES)], axis=0)
    return y.astype(np.float32)



# revision 2
# speedup vs baseline: 1.1941x; 1.1941x over previous
"""Trainium2 Bass kernel v2 for the char-GRU (transposed-gates design).

  y = FC(GRU_last_hidden(Embed(x)))   V=128, E=H=OUT=768, B=128, T=512.

Data-parallel over batch: 8 cores x 16 rows. Per core:

  - All weight matmuls run with the WEIGHTS as the 128x128 stationary
    operand and the state h^T as the 16-col moving operand, so gates come
    out TRANSPOSED: [gate_dim (partition), batch].  FWL makes the
    per-tile LDWEIGHTS cheap (fp8: ~27-53ns).
  - x-side: table[v,:] = emb[v] @ W_ih.T + b_ih (+ b_hh on r/z cols),
    gathered via one-hot MOVING operand against stationary table tiles,
    batched 4 steps per LDW set, accumulated directly into the gate PSUM.
  - Gate chain runs on [128, 96] tiles (128-partition elementwise),
    producing h^T directly -- no per-step transposes.
  - h^T stored bf16 (single copy, feeds both matmul + blend).
"""

import numpy as np
from contextlib import ExitStack

import concourse.bass as bass
import concourse.bacc as bacc
import concourse.tile as tile
from concourse import mybir
from concourse.bass_utils import run_bass_kernel_spmd

F32 = mybir.dt.float32
BF16 = mybir.dt.bfloat16
FP8 = mybir.dt.float8e4

V, E, H, OUT = 128, 768, 768, 768
G3 = 3 * H
B_FULL, T_FULL = 128, 512
NCORES = 8
BS = B_FULL // NCORES   # 16
KT = H // 128           # 6
NSLOT = 4               # gather batch (steps per psum group)

W_DT = FP8              # recurrent weight dtype (stationary)
H_DT = BF16             # h^T state dtype (moving)
DEBUG = False           # add step-0 intermediate dumps


def emit_kernel(ctx: ExitStack, tc: tile.TileContext, io: dict, T: int,
                REPS: int = 1):
    nc = tc.nc
    add = mybir.AluOpType.add
    mult = mybir.AluOpType.mult
    iseq = mybir.AluOpType.is_equal
    Sig = mybir.ActivationFunctionType.Sigmoid
    Tanh = mybir.ActivationFunctionType.Tanh
    Ident = mybir.ActivationFunctionType.Identity

    NSLOT = min(4, T)  # noqa: local shadow for small-T tests
    assert T % NSLOT == 0
    NG = T // NSLOT  # number of gather groups

    consts = ctx.enter_context(tc.tile_pool(name="consts", bufs=1))
    whh_sb = consts.tile([128, KT, G3], W_DT, name="whh_sb")
    table_sb = consts.tile([128, G3], BF16, name="table_sb")
    oh_sb = consts.tile([128, T * BS], BF16, name="oh_sb")
    bhn_sb = consts.tile([128, KT, BS], F32, name="bhn_sb")
    fcw_sb = consts.tile([128, KT, OUT], BF16, name="fcw_sb")
    fcb_sb = consts.tile([128, KT], F32, name="fcb_sb")
    ones128 = consts.tile([128, 128], F32, name="ones128")
    ident128 = consts.tile([128, 128], F32, name="ident128")

    state = ctx.enter_context(tc.tile_pool(name="state", bufs=1))
    hT = state.tile([128, KT, BS], H_DT, name="hT")
    hT8 = state.tile([128, KT, BS], FP8, name="hT8")

    tmp = ctx.enter_context(tc.tile_pool(name="tmp", bufs=2))
    # gather-group psum (r/z/xn classes, NSLOT steps each), ping-pong
    psg = ctx.enter_context(tc.tile_pool(name="psg", bufs=2, space="PSUM"))
    # per-step hn psum + FC tail
    psh = ctx.enter_context(tc.tile_pool(name="psh", bufs=2, space="PSUM"))

    def emit_init():
        nc.sync.dma_start(table_sb[:], io["table"][:])
        nc.sync.dma_start(bhn_sb[:], io["bhn"][:])
        nc.sync.dma_start(fcb_sb[:], io["fcb"][:])
        for k in range(KT):
            nc.sync.dma_start(whh_sb[:, k, :], io["whh"][k])
            nc.sync.dma_start(fcw_sb[:, k, :], io["fcw"][k])
        nq = 4
        step = (T * BS) // nq
        for q in range(nq):
            nc.sync.dma_start(oh_sb[:, q * step:(q + 1) * step],
                              io["oh"][:, q * step:(q + 1) * step])
        nc.vector.memset(ones128[:], 1.0)
        nc.gpsimd.affine_select(ident128[:], ones128[:], pattern=[[1, 128]],
                                compare_op=iseq, fill=0.0, base=0,
                                channel_multiplier=-1)
        nc.vector.memset(hT[:], 0.0)
        nc.vector.memset(hT8[:], 0.0)

    # table col ranges: class c (0=r,1=z,2=n/x) tile j covers
    # gate dims c*768 + j*128 ...  +128
    def gcol(c, j):
        return c * H + j * 128

    def emit_gather_group(g, ps_r, ps_z, ps_x, tiles):
        """Gather MMs for group g (steps 4g..4g+3) for the given tile js.
        psum class layout: [128, KT, NSLOT*BS] -> gather writes [:, j, :]
        (contiguous 64), kloop writes [:, j, slot*BS:+BS] (contiguous 16)."""
        c0 = g * NSLOT * BS
        mv = oh_sb[:, c0:c0 + NSLOT * BS]
        for (c, j, dst) in tiles:
            st = table_sb[:, gcol(c, j):gcol(c, j) + 128]
            # start=True only on the bank's first write (j==0): start marks
            # the WHOLE 2KB zero-region pending; later writes then overwrite
            # on first touch and accumulate after.
            nc.tensor.matmul(dst[:, j, :], st, mv,
                             start=(j == 0), stop=False, skip_group_check=True)

    def alloc_group():
        ps_r = psg.tile([128, KT, NSLOT * BS], F32, name="ps_r", tag="gr")
        ps_z = psg.tile([128, KT, NSLOT * BS], F32, name="ps_z", tag="gz")
        ps_x = psg.tile([128, KT, NSLOT * BS], F32, name="ps_x", tag="gx")
        return ps_r, ps_z, ps_x

    def emit_body():
        emit_init()
        groups = {}
        groups[0] = alloc_group()
        emit_gather_group(0, *groups[0],
                          [(c, j, groups[0][c]) for c in range(3)
                           for j in range(KT)])

        for t in range(T):
            g, slot = t // NSLOT, t % NSLOT
            if slot == 0 and g + 1 < NG:
                groups[g + 1] = alloc_group()
            ps_r, ps_z, ps_x = groups[g]
            ps_hn = psh.tile([128, KT, BS], F32, name="ps_hn", tag="hn")

            # spread next group's gather tiles across this group's steps
            if g + 1 < NG:
                ntiles = [(c, j) for c in range(3) for j in range(KT)]
                lo = (slot * 18) // NSLOT
                hi = ((slot + 1) * 18) // NSLOT
                nxt = groups[g + 1]
                emit_gather_group(g + 1, *nxt,
                                  [(c, j, nxt[c]) for (c, j) in ntiles[lo:hi]])

            # recurrent matmuls.  hn: j-outer (one psum group at a time per
            # bank); r/z: k-outer accumulation onto the gather groups.
            for j in range(KT):
                for kt in range(KT):
                    st = whh_sb[:, kt, gcol(2, j):gcol(2, j) + 128]
                    nc.tensor.matmul(ps_hn[:, j, :], st, hT8[:, kt, :],
                                     start=(j == 0 and kt == 0),
                                     stop=(j == KT - 1 and kt == KT - 1),
                                     skip_group_check=True)
            for kt in range(KT):
                mv = hT8[:, kt, :]
                for c in (0, 1):
                    for j in range(KT):
                        st = whh_sb[:, kt, gcol(c, j):gcol(c, j) + 128]
                        dst = ps_r if c == 0 else ps_z
                        out = dst[:, j, slot * BS:(slot + 1) * BS]
                        nc.tensor.matmul(out, st, mv, start=False,
                                         stop=(kt == KT - 1),
                                         skip_group_check=True)

            # gate chain, [128, 96] ops
            r_t = tmp.tile([128, KT, BS], F32, name="r_t", tag="r")
            z_t = tmp.tile([128, KT, BS], F32, name="z_t", tag="z")
            u_t = tmp.tile([128, KT, BS], F32, name="u_t", tag="u")
            hnb = tmp.tile([128, KT, BS], F32, name="hnb", tag="hnb")
            a_t = tmp.tile([128, KT, BS], F32, name="a_t", tag="a")
            b_t = tmp.tile([128, KT, BS], F32, name="b_t", tag="b")
            n_t = tmp.tile([128, KT, BS], F32, name="n_t", tag="n")
            d1_t = tmp.tile([128, KT, BS], F32, name="d1_t", tag="d1")
            d2_t = tmp.tile([128, KT, BS], F32, name="d2_t", tag="d2")
            sl = slice(slot * BS, (slot + 1) * BS)

            nc.vector.tensor_tensor(hnb[:], ps_hn[:], bhn_sb[:], add)
            nc.scalar.activation(r_t[:], ps_r[:, :, sl], Sig)
            nc.scalar.activation(z_t[:], ps_z[:, :, sl], Sig)
            nc.gpsimd.tensor_tensor(d2_t[:], z_t[:], hT[:], mult)
            nc.scalar.activation(u_t[:], ps_z[:, :, sl], Sig, scale=-1.0)
            nc.gpsimd.tensor_tensor(a_t[:], r_t[:], hnb[:], mult)
            nc.vector.tensor_tensor(b_t[:], a_t[:], ps_x[:, :, sl], add)
            nc.scalar.activation(n_t[:], b_t[:], Tanh)
            nc.vector.tensor_tensor(d1_t[:], u_t[:], n_t[:], mult)
            nc.vector.tensor_tensor(hT[:], d1_t[:], d2_t[:], add)
            nc.scalar.copy(hT8[:], hT[:])
            if DEBUG and t == 0:
                dbg_x = tmp.tile([128, KT, BS], F32, name="dbg_x", tag="dbx")
                nc.vector.tensor_scalar_add(dbg_x[:], ps_x[:, :, sl], 0.0)
                nc.sync.dma_start(io["dbg_x"][:], dbg_x[:])
                dbg_rp = tmp.tile([128, KT, BS], F32, name="dbg_rp", tag="dbr")
                nc.vector.tensor_scalar_add(dbg_rp[:], ps_r[:, :, sl], 0.0)
                nc.sync.dma_start(io["dbg_rp"][:], dbg_rp[:])
                nc.sync.dma_start(io["dbg_r"][:], r_t[:])
                nc.sync.dma_start(io["dbg_n"][:], n_t[:])
                nc.sync.dma_start(io["dbg_h"][:], d1_t[:])

        # ---- FC head: yT = fcW @ h + fcb, then transpose back ----
        ps_y = psh.tile([128, KT * BS], F32, name="ps_y", tag="hn")
        for ot in range(KT):
            for kt in range(KT):
                st = fcw_sb[:, kt, ot * 128:(ot + 1) * 128]
                nc.tensor.matmul(ps_y[:, ot * BS:(ot + 1) * BS], st,
                                 hT[:, kt, :],
                                 start=(ot == 0 and kt == 0),
                                 stop=(ot == KT - 1 and kt == KT - 1),
                                 skip_group_check=True)
        yT_sb = tmp.tile([128, KT * BS], F32, name="yT_sb", tag="r")
        for ot in range(KT):
            nc.scalar.activation(yT_sb[:, ot * BS:(ot + 1) * BS],
                                 ps_y[:, ot * BS:(ot + 1) * BS], Ident,
                                 bias=fcb_sb[:, ot:ot + 1])
        y_sb = tmp.tile([BS, OUT], F32, name="y_sb", tag="z")
        for ot in range(KT):
            ps_t = psh.tile([BS, 128], F32, name="ps_t", tag="hn")
            nc.tensor.transpose(ps_t[:], yT_sb[:, ot * BS:(ot + 1) * BS],
                                ident128[:])
            nc.scalar.copy(y_sb[:, ot * 128:(ot + 1) * 128], ps_t[:])
        nc.sync.dma_start(io["y"][:], y_sb[:])

    if REPS == 1:
        emit_body()
    else:
        with tc.For_i(0, REPS, 1):
            emit_body()


def build(T: int = T_FULL, num_devices: int = NCORES, reps: int = 1):
    nc = bacc.Bacc("TRN2", target_bir_lowering=False, debug=False,
                   enable_asserts=False, num_devices=num_devices)
    io = {
        "oh": nc.dram_tensor("oh", [128, T * BS], BF16,
                             kind="ExternalInput").ap(),
        "whh": nc.dram_tensor("whh", [KT, 128, G3], W_DT,
                              kind="ExternalInput").ap(),
        "table": nc.dram_tensor("table", [128, G3], BF16,
                                kind="ExternalInput").ap(),
        "bhn": nc.dram_tensor("bhn", [128, KT * BS], F32,
                              kind="ExternalInput").ap(),
        "fcw": nc.dram_tensor("fcw", [KT, 128, OUT], BF16,
                              kind="ExternalInput").ap(),
        "fcb": nc.dram_tensor("fcb", [128, KT], F32,
                              kind="ExternalInput").ap(),
        "y": nc.dram_tensor("y", [BS, OUT], F32, kind="ExternalOutput").ap(),
    }
    if DEBUG:
        for nm in ("dbg_x", "dbg_rp", "dbg_r", "dbg_n", "dbg_h"):
            io[nm] = nc.dram_tensor(nm, [128, KT, BS], F32,
                                    kind="ExternalOutput").ap()
    with tile.TileContext(nc) as tc, ExitStack() as ctx:
        emit_kernel(ctx, tc, io, T, REPS=reps)
    nc.compile()
    return nc


def make_in_maps(x, emb, W_ih, W_hh, b_ih, b_hh, fc_W, fc_b,
                 T: int = T_FULL, ncores: int = NCORES):
    w_np = mybir.dt.np(W_DT)
    x = np.asarray(x).astype(np.int32)[:, :T]
    emb = np.asarray(emb, np.float32)
    W_ih = np.asarray(W_ih, np.float32)
    W_hh = np.asarray(W_hh, np.float32)
    b_ih = np.asarray(b_ih, np.float32)
    b_hh = np.asarray(b_hh, np.float32)
    fc_W = np.asarray(fc_W, np.float32)
    fc_b = np.asarray(fc_b, np.float32)

    table = emb @ W_ih.T + b_ih
    table[:, :2 * H] += b_hh[:2 * H]
    table = table.astype(mybir.dt.np(BF16))                      # [V, 3H]
    # whh[kt][p, g] = W_hh[g, kt*128+p]
    whh = np.ascontiguousarray(
        W_hh.T.reshape(KT, 128, G3)).astype(w_np)                # [KT,128,G3]
    bhn = np.repeat(
        b_hh[2 * H:].reshape(KT, 128).T[:, :, None], BS,
        axis=2).reshape(128, KT * BS).astype(np.float32)         # [128,KT*BS]
    fcw = np.ascontiguousarray(
        fc_W.T.reshape(KT, 128, OUT)).astype(mybir.dt.np(BF16))  # [KT,128,OUT]
    fcb = np.ascontiguousarray(
        fc_b.reshape(KT, 128).T).astype(np.float32)              # [128, KT]

    shared = {"whh": whh, "table": table, "bhn": bhn, "fcw": fcw, "fcb": fcb}
    cols = np.arange(T * BS)
    in_maps = []
    for c in range(ncores):
        xs = x[c * BS:(c + 1) * BS]                          # [BS, T]
        x_tmaj = np.ascontiguousarray(xs.T).reshape(T * BS)  # t-major
        oh = np.zeros((V, T * BS), np.float32)
        oh[x_tmaj, cols] = 1.0
        in_maps.append({"oh": oh.astype(mybir.dt.np(BF16)), **shared})
    return in_maps


_CACHE = {}


def kernel(x, emb, W_ih, W_hh, b_ih, b_hh, fc_W, fc_b):
    if "nc" not in _CACHE:
        _CACHE["nc"] = build()
    nc = _CACHE["nc"]
    in_maps = make_in_maps(x, emb, W_ih, W_hh, b_ih, b_hh, fc_W, fc_b)
    res = run_bass_kernel_spmd(nc, in_maps, core_ids=list(range(NCORES)))
    y = np.concatenate([res.results[c]["y"] for c in range(NCORES)], axis=0)
    return y.astype(np.float32)
